# revision 42
# baseline (speedup 1.0000x reference)
"""Trainium2 Bass kernel for nn_Attention_54030688584207.

Single-head attention block:
    h = LN(x^T) ; qkv = h @ W^T + b ; S = q k^T / sqrt(N) + position
    out = softmax(S) @ v, returned as [B, C, N].

Sharding: 8 cores = 4 batches x 2 query-halves, no collectives. Each core
receives its batch's x rotated so its own 1024 query tokens come first and
computes q for its half plus full K/V for the batch (K/V replicated within
the pair), then scores/softmax/PV for its 1024 query rows.

LayerNorm is folded into the QKV epilogues instead of materializing h:
    qkv[d,n] = rstd[n]*( (W'x)[d,n] - mu[n]*wsum[d] ) + b'[d]
so all projection matmuls run on raw (bf16) x with no LN dependency. The
LN statistics are computed entirely off the tensor engine: per-512-token
chunk, the 8 channel-chunks of x (and of x^2 from ScalarE squares) are
pair-summed on the DVE (bf16, multi-dim APs, in place), then reduced
across the 128 partitions with a GpSimd partition_all_reduce, giving
full-width [128,512] sums with no matmul and no broadcast. rstd comes
from DVE reciprocal + ScalarE Sqrt (no Ln -> no activation-table thrash).

Softmax skips max-subtraction (scores are O(5), safe in f32/bf16) so
exp(S^T) feeds PV directly as the stationary operand; row sums accumulate
in a single PSUM bank ([128,8], one column per query block) via 1-wide
matmuls folded into phase C, so phase D starts with all reciprocals ready
and the kernel tail is one epilogue + DMA.

Device layouts (per core):
    x_sh  [C=1024, N=2048] bf16  channels x tokens (token-rotated)
    w_t   [C=1024, 3C=3072] bf16 W'^T (gamma/SCALE folded on host)
    bias  [3072] f32             b' (beta folded, q-part scaled)
    pos_t [N=2048, MY=1024] bf16 position^T (rows in local key order)
    out   [MY=1024, C=1024] f32  out[i, c]  (host transposes back)
"""

import os
import sys

for _p in ("/opt/trn_rl_repo",):
    if _p not in sys.path and os.path.isdir(_p):
        sys.path.insert(0, _p)

import numpy as np
import ml_dtypes

import concourse.bass as bass
import concourse.bass_isa as bass_isa
import concourse.tile as tile
from concourse import bacc, mybir
from concourse.bass import ts, ds
from concourse.bass_utils import run_bass_kernel_spmd

FP = mybir.dt.float32
BF = mybir.dt.bfloat16
AF = mybir.ActivationFunctionType

B = 4
C = 1024
N = 2048
MY = 1024  # query rows per core
D3 = 3 * C
NCH = C // 128   # 8 channel chunks
NJT = N // 128   # 16 key tiles
NIB = MY // 128  # 8 query blocks
NTC = N // 512   # 4 token chunks
LN_EPS = 1e-5
SCALE = 1.0 / np.sqrt(N)


def build_kernel(rep=1, qk_bias=False, v_bias=False):
    nc = bacc.Bacc("TRN2", target_bir_lowering=False, debug=False, num_devices=8)
    x_ext = nc.declare_dram_parameter("x_sh", [C, N], BF, isOutput=False)
    wt_ext = nc.declare_dram_parameter("w_t", [C, D3], BF, isOutput=False)
    b_ext = nc.declare_dram_parameter("bias", [D3], FP, isOutput=False)
    ws_ext = nc.declare_dram_parameter("wsum", [D3], FP, isOutput=False)
    pos_ext = nc.declare_dram_parameter("pos_t", [N, MY], BF, isOutput=False)
    out_ext = nc.declare_dram_parameter("out", [MY, C], BF, isOutput=True)

    x_r = x_ext.ap().rearrange("(a p) n -> p a n", p=128)      # [128, 8, N]
    wt_r = wt_ext.ap().rearrange("(a p) d -> p a d", p=128)    # [128, 8, D3]
    b_r = b_ext.ap().rearrange("(a p) -> p a", p=128)          # [128, 24]
    ws_r = ws_ext.ap().rearrange("(a p) -> p a", p=128)        # [128, 24]

    with tile.TileContext(nc) as tc:
      for _r in range(rep):
        with (
            tc.tile_pool(name=f"res{_r}", bufs=1) as res,
            tc.tile_pool(name=f"statb{_r}", bufs=2) as statb,
            tc.tile_pool(name=f"pospool{_r}", bufs=2) as pospool,
            tc.tile_pool(name=f"xsqp{_r}", bufs=1) as xsqp,
            tc.tile_pool(name=f"treep{_r}", bufs=1) as treep,
            tc.tile_pool(name=f"sump{_r}", bufs=2) as sump,
            tc.tile_pool(name=f"scr{_r}", bufs=3) as scr,
            tc.tile_pool(name=f"rows{_r}", bufs=1) as rows,
            tc.tile_pool(name=f"dramp{_r}", bufs=1, space="DRAM") as dramp,
            tc.tile_pool(name=f"psum{_r}", bufs=1, space="PSUM") as psum,
        ):
            # ---- resident tiles ----
            xh = res.tile([128, NCH, N], BF, tag="big")       # raw x (bf16)
            qs = res.tile([128, NCH, MY], BF, tag="qs")       # q^T  [c, i]
            ks = res.tile([128, NCH, N], BF, tag="ks")        # k^T  [c, j]
            vs = res.tile([128, NJT, C], BF, tag="vs")        # v    [j, c]
            wqk = res.tile([128, NCH, 2 * C], BF, tag="wqk")  # W'^T q,k cols
            wv = res.tile([128, NCH, C], BF, tag="wv")        # W'^T v cols

            ones_b = rows.tile([128, 1], BF, tag="ones_b")
            nc.vector.memset(ones_b[:], 1.0)

            # LN stat tiles (bf16, full width): -mu*rstd and rstd per token
            nmr_b = statb.tile([128, N], BF, tag="statmb", name="nmr_b")
            rstd_b = statb.tile([128, N], BF, tag="statmb", name="rstd_b")
            # per-token-tile columns for the v epilogue (via DRAM bounce):
            # -mu and +rstd (f32)
            nmu_col = rows.tile([128, NJT], FP, tag="nmu_col")
            rstd_col = rows.tile([128, NJT], FP, tag="rstd_col")
            nmu_dram = dramp.tile([1, N], FP, tag="nmu_dram")
            rstd_dram = dramp.tile([1, N], FP, tag="rstd_dram")

            # ---- load x and weights (x t0 first: stats matmuls chase it) ----
            for ch in range(4):
                nc.sync.dma_start(xh[:, ds(ch * 2, 2), ts(0, 512)],
                                  x_r[:, ds(ch * 2, 2), ts(0, 512)])
            nc.sync.dma_start(wqk[:, :, ds(0, 256)], wt_r[:, :, ds(0, 256)])
            nc.sync.dma_start(wqk[:, :, ds(256, 256)], wt_r[:, :, ds(256, 256)])
            bias_sb = rows.tile([128, 24], FP, tag="bias")
            nc.sync.dma_start(bias_sb[:], b_r)
            wsum_sb = rows.tile([128, 24], FP, tag="wsum")
            nc.sync.dma_start(wsum_sb[:], ws_r)
            nc.sync.dma_start(wqk[:, :, ds(512, 512)], wt_r[:, :, ds(512, 512)])
            nc.sync.dma_start(xh[:, :, ts(1, 512)], x_r[:, :, ts(1, 512)])
            nc.sync.dma_start(xh[:, :, ts(2, 512)], x_r[:, :, ts(2, 512)])
            nc.sync.dma_start(xh[:, :, ts(3, 512)], x_r[:, :, ts(3, 512)])
            for piece in range(2):
                nc.sync.dma_start(wqk[:, :, ds(C + piece * 512, 512)],
                                  wt_r[:, :, ds(C + piece * 512, 512)])
            nc.sync.dma_start(wv[:], wt_r[:, :, ds(2 * C, C)])

            # v-weight-colsum (+opt bias) broadcast rows [1, C] -> [128, C]
            wvrow = statb.tile([1, C], BF, tag="statb", name="wvrow")
            nc.gpsimd.dma_start(wvrow[:], ws_ext.ap()[ds(2 * C, C)].rearrange("(o c) -> o c", o=1))
            wvsum_b = rows.tile([128, C], BF, tag="wvsb")
            nc.gpsimd.partition_broadcast(wvsum_b[:], wvrow[:])
            if v_bias:
                bvrow = statb.tile([1, C], BF, tag="statb", name="bvrow")
                nc.gpsimd.dma_start(bvrow[:], b_ext.ap()[ds(2 * C, C)].rearrange("(o c) -> o c", o=1))
                bv_b = rows.tile([128, C], BF, tag="bvb")
                nc.gpsimd.partition_broadcast(bv_b[:], bvrow[:])

            # ---- Phase A: LN stats per 512-token chunk ----
            # t0 on the PE (ones-matmuls chase the x DMA and warm the ramp);
            # t1-3 off the PE: x pair-sum tree on DVE, x^2 tree on GpSimd,
            # then a partition_all_reduce gives full-width sums directly.
            def stats_cols(t, nrow, rrow):
                    nc.sync.dma_start(nmu_dram[0:1, ts(t, 512)], nrow[:])
                    nc.sync.dma_start(rstd_dram[0:1, ts(t, 512)], rrow[:])
                    nc.sync.dma_start(
                        nmu_col[:, ds(t * 4, 4)],
                        nmu_dram[0:1, ts(t, 512)].rearrange("o (f p) -> (o p) f", p=128))
                    nc.sync.dma_start(
                        rstd_col[:, ds(t * 4, 4)],
                        rstd_dram[0:1, ts(t, 512)].rearrange("o (f p) -> (o p) f", p=128))

            def stats_rows_psum(t, sx_row, sq_ap):
                    # [1,512] row chain from PSUM sums, then Pool broadcasts:
                    # nmu = -sx/C ; var = C*var_true = sq - C*nmu^2
                    # rstd = sqrt(C * 1/var)
                    nrow = scr.tile([1, 512], FP, tag="row", bufs=2,
                                    name=f"nrow{t}")
                    nc.scalar.mul(nrow[:], sx_row[:], -1.0 / C)
                    var = scr.tile([1, 512], FP, tag="var", bufs=1,
                                   name=f"varr{t}")
                    nc.vector.tensor_mul(var[:], nrow[:], nrow[:])
                    nc.vector.scalar_tensor_tensor(
                        var[:], var[:], -float(C), sq_ap,
                        op0=mybir.AluOpType.mult, op1=mybir.AluOpType.add)
                    nc.vector.reciprocal(var[:], var[:])
                    rrow = scr.tile([1, 512], FP, tag="row", bufs=2,
                                    name=f"rrow{t}")
                    nc.scalar.activation(rrow[:], var[:], AF.Sqrt,
                                         scale=float(C))
                    rstd_cb = scr.tile([1, 512], BF, tag="cb", bufs=2,
                                       name=f"rstd_cb{t}")
                    nc.vector.tensor_copy(rstd_cb[:], rrow[:])
                    nmr_cb = scr.tile([1, 512], BF, tag="cb", bufs=2,
                                      name=f"nmr_cb{t}")
                    nc.vector.tensor_mul(nmr_cb[:], nrow[:], rrow[:])
                    nc.gpsimd.partition_broadcast(rstd_b[:, ts(t, 512)],
                                                  rstd_cb[:])
                    nc.gpsimd.partition_broadcast(nmr_b[:, ts(t, 512)],
                                                  nmr_cb[:])
                    stats_cols(t, nrow, rrow)

            def stats_rows_sbuf(t, sx_row, sq_row):
                    # full-width chain on [128,512] SBUF sums (no broadcast):
                    # var = reciprocal(sx*sx/C - sq) = -1/(C*var_true)
                    # rstd = sqrt(var * -C) ; nmr = (sx * -1/C) * rstd
                    var = scr.tile([128, 512], FP, tag="var", bufs=1,
                                   name=f"var{t}")
                    nc.vector.tensor_mul(var[:], sx_row[:], sx_row[:])
                    nc.vector.scalar_tensor_tensor(
                        var[:], var[:], 1.0 / C, sq_row[:],
                        op0=mybir.AluOpType.mult, op1=mybir.AluOpType.subtract)
                    nc.vector.reciprocal(var[:], var[:])
                    nc.scalar.activation(rstd_b[:, ts(t, 512)], var[:],
                                         AF.Sqrt, scale=-float(C))
                    nc.vector.scalar_tensor_tensor(
                        nmr_b[:, ts(t, 512)], sx_row[:], -1.0 / C,
                        rstd_b[:, ts(t, 512)],
                        op0=mybir.AluOpType.mult, op1=mybir.AluOpType.mult)
                    nrow = scr.tile([1, 512], FP, tag="row", bufs=2,
                                    name=f"nrow{t}")
                    nc.scalar.mul(nrow[:], sx_row[0:1, :], -1.0 / C)
                    rrow = scr.tile([1, 512], FP, tag="row", bufs=2,
                                    name=f"rrow{t}")
                    nc.scalar.activation(rrow[:], var[0:1, :], AF.Sqrt,
                                         scale=-float(C))
                    stats_cols(t, nrow, rrow)

            def stats_chunk_pe(t):
                    # raw-x column sums on the PE: 8 dense ap-512 matmuls that
                    # chase the x DMA and anchor the tensor engine's p-state
                    # ramp; the x^2 sums go through the DVE tree (their ACT
                    # squares would gate the matmul group too late anyway).
                    ps_s = psum.tile([1, 512], FP, tag="w", bufs=7, name=f"ps_s{t}")
                    for c in range(NCH):
                        nc.tensor.matmul(ps_s[:], ones_b[:], xh[:, c, ts(t, 512)],
                                         start=(c == 0), stop=(c == NCH - 1))
                    sum_q = stats_sums_q(t)
                    stats_rows_psum(t, ps_s, sum_q[0:1, :])

            def stats_sums_x(t):
                    xt = treep.tile([128, 4, 512], BF, tag="xt", name=f"xt{t}")
                    nc.vector.tensor_add(xt[:], xh[:, ds(0, 4), ts(t, 512)],
                                         xh[:, ds(4, 4), ts(t, 512)])
                    nc.vector.tensor_add(xt[:, ds(0, 2), :], xt[:, ds(0, 2), :],
                                         xt[:, ds(2, 2), :])
                    sum_x = sump.tile([128, 512], FP, tag="sum_x", bufs=1,
                                      name=f"sum_x{t}")
                    nc.vector.tensor_add(sum_x[:], xt[:, 0, :], xt[:, 1, :])
                    nc.gpsimd.partition_all_reduce(sum_x[:], sum_x[:], 128,
                                                   bass_isa.ReduceOp.add)
                    return sum_x

            def stats_sums_q(t):
                    qt = treep.tile([128, 4, 512], BF, tag="qt", name=f"qt{t}")
                    xsq = xsqp.tile([128, 4, 512], BF, tag="xsq", name=f"xsq{t}")
                    for c in range(4):
                        nc.scalar.square(qt[:, ds(c, 1), :],
                                         xh[:, ds(c, 1), ts(t, 512)])
                    for c in range(4):
                        nc.scalar.square(xsq[:, ds(c, 1), :],
                                         xh[:, ds(c + 4, 1), ts(t, 512)])
                    nc.vector.tensor_add(qt[:], qt[:], xsq[:])
                    nc.vector.tensor_add(qt[:, ds(0, 2), :], qt[:, ds(0, 2), :],
                                         qt[:, ds(2, 2), :])
                    sum_q = sump.tile([128, 512], FP, tag="sum_q", bufs=1,
                                      name=f"sum_q{t}")
                    nc.vector.tensor_add(sum_q[:], qt[:, 0, :], qt[:, 1, :])
                    nc.gpsimd.partition_all_reduce(sum_q[:], sum_q[:], 128,
                                                   bass_isa.ReduceOp.add)
                    return sum_q

            def stats_chunk(t):
                    sum_x = stats_sums_x(t)
                    sum_q = stats_sums_q(t)
                    stats_rows_sbuf(t, sum_x, sum_q)

            # ---- Phase B1: q^T and k^T (weights stationary, c-outer groups) ----
            def qk_group(dts, tlist):
                    pss = {}
                    for dt in dts:
                        for t in tlist:
                            pss[(dt, t)] = psum.tile([128, 512], FP, tag="w",
                                                     bufs=7, name=f"qkv_{dt}_{t}")
                    for c in range(NCH):
                        for dt in dts:
                            for t in tlist:
                                nc.tensor.matmul(
                                    pss[(dt, t)][:], wqk[:, c, ts(dt, 128)],
                                    xh[:, c, ts(t, 512)],
                                    start=(c == 0), stop=(c == NCH - 1))
                    for dt in dts:
                        for t in tlist:
                            # t1 = G + (-mu*rstd)*wsum[d] ; qk = t1 * rstd
                            t1 = scr.tile([128, 512], BF, tag="t1", bufs=2,
                                          name=f"t1_{dt}_{t}")
                            nc.vector.scalar_tensor_tensor(
                                t1[:], nmr_b[:, ts(t, 512)], wsum_sb[:, dt:dt + 1],
                                pss[(dt, t)][:],
                                op0=mybir.AluOpType.mult, op1=mybir.AluOpType.add)
                            dst = (qs[:, dt, ts(t, 512)] if dt < 8
                                   else ks[:, dt - 8, ts(t, 512)])
                            nc.vector.tensor_mul(dst, t1[:], rstd_b[:, ts(t, 512)])
                            if qk_bias:
                                nc.vector.tensor_scalar_add(
                                    dst, dst, bias_sb[:, dt:dt + 1])

            stats_chunk_pe(0)
            qk_group(range(0, 2), [0])
            qk_group(range(2, 4), [0])
            stats_parts = {}
            stats_parts["x1"] = stats_sums_x(1)
            qk_group(range(4, 6), [0])
            stats_parts["q1"] = stats_sums_q(1)
            qk_group(range(6, 8), [0])
            stats_rows_sbuf(1, stats_parts["x1"], stats_parts["q1"])
            for g in range(0, 8, 2):
                qk_group(range(g, g + 2), [1])

            # ---- Phase B2: v (activations stationary) ----
            def v_group(jts):
                for jt in jts:
                    for cc in range(C // 512):
                        psv = psum.tile([128, 512], FP, tag="w",
                                        bufs=7, name=f"psv_{jt}_{cc}")
                        for c in range(NCH):
                            nc.tensor.matmul(
                                psv[:], xh[:, c, ts(jt, 128)],
                                wv[:, c, ts(cc, 512)],
                                start=(c == 0), stop=(c == NCH - 1))
                        # t1 = Gv + wvsum*(-mu[n]) ; v = t1*rstd[n] (+ bv)
                        t1v = scr.tile([128, 512], BF, tag="t1", bufs=2,
                                       name=f"t1v_{jt}_{cc}")
                        nc.vector.scalar_tensor_tensor(
                            t1v[:], wvsum_b[:, ts(cc, 512)], nmu_col[:, jt:jt + 1],
                            psv[:],
                            op0=mybir.AluOpType.mult, op1=mybir.AluOpType.add)
                        if v_bias:
                            nc.vector.scalar_tensor_tensor(
                                vs[:, jt, ts(cc, 512)], t1v[:],
                                rstd_col[:, jt:jt + 1], bv_b[:, ts(cc, 512)],
                                op0=mybir.AluOpType.mult, op1=mybir.AluOpType.add)
                        else:
                            nc.scalar.mul(vs[:, jt, ts(cc, 512)], t1v[:],
                                          rstd_col[:, jt:jt + 1])

            # k for tokens 0:1024 first (epilogues need only stats 0/1);
            # the stats 2/3 DVE bursts spread across this PE work, then
            # k for tokens 1024:2048 and v interleave.
            for g in range(8, 16):
                qk_group(range(g, g + 1), [0, 1])
                if g == 8:
                    stats_parts["x2"] = stats_sums_x(2)
                elif g == 9:
                    stats_parts["q2"] = stats_sums_q(2)
                elif g == 10:
                    stats_rows_sbuf(2, stats_parts["x2"], stats_parts["q2"])
                elif g == 11:
                    stats_parts["x3"] = stats_sums_x(3)
                elif g == 12:
                    stats_parts["q3"] = stats_sums_q(3)
                elif g == 13:
                    stats_rows_sbuf(3, stats_parts["x3"], stats_parts["q3"])
            for gi, g in enumerate(range(8, 16)):
                qk_group(range(g, g + 1), [2, 3])
                if g % 2 == 1:
                    v_group(range((g - 9) // 2 * 4, (g - 9) // 2 * 4 + 4))

            # ---- Phase C: S^T = k^T.T q^T + pos ; exp -> es (bf16) ----
            # row sums fold in as 1-wide matmuls into one PSUM bank [128, 8]
            es = res.tile([128, NJT, MY], BF, tag="big")  # reuses xh slot
            ps_sums = psum.tile([128, NIB], FP, tag="sums", bufs=1,
                                name="ps_sums")
            for j in range(NJT):
                    pos_tile = pospool.tile([128, MY], BF, tag="pos")
                    nc.sync.dma_start(pos_tile[:], pos_ext[ts(j, 128), :])
                    psS = [psum.tile([128, 512], FP, tag="w", bufs=7,
                                     name=f"psS{j}_{ih}") for ih in range(2)]
                    for c in range(NCH):
                        for ih in range(MY // 512):
                            nc.tensor.matmul(
                                psS[ih][:], ks[:, c, ts(j, 128)],
                                qs[:, c, ts(ih, 512)],
                                start=(c == 0), stop=(c == NCH - 1))
                    if j > 0:
                        # single start=True: PSUM "start" begins the whole
                        # 2KB zero-region; later writes to untouched elements
                        # overwrite via per-element has_written
                        for i in range(NIB):
                            nc.tensor.matmul(
                                ps_sums[:, i:i + 1], es[:, j - 1, ts(i, 128)],
                                ones_b[:], start=(j == 1 and i == 0),
                                stop=False)
                    for ih in range(2):
                        nc.vector.tensor_add(psS[ih][:], psS[ih][:],
                                             pos_tile[:, ts(ih, 512)])
                        nc.scalar.activation(es[:, j, ts(ih, 512)], psS[ih][:],
                                             AF.Exp)

            # ---- Phase D: out[i, c] = (P^T)^T v / rowsum ----
            recips = rows.tile([128, NIB], FP, tag="recips")
            for i in range(NIB):
                    pso = [psum.tile([128, 512], FP, tag="w", bufs=7,
                                     name=f"pso{i}_{cc}") for cc in range(2)]
                    for j in range(NJT):
                        nc.tensor.matmul(
                            pso[0][:], es[:, j, ts(i, 128)], vs[:, j, ts(0, 512)],
                            start=(j == 0), stop=(j == NJT - 1))
                    if i == 0:
                        # last rowsum tile (es j=15), then all reciprocals
                        for ii in range(NIB):
                            nc.tensor.matmul(
                                ps_sums[:, ii:ii + 1],
                                es[:, NJT - 1, ts(ii, 128)],
                                ones_b[:], start=False, stop=(ii == NIB - 1))
                        nc.vector.reciprocal(recips[:], ps_sums[:])
                    out_t = statb.tile([128, C], BF, tag="statb", name=f"out_t{i}")
                    # cc0 epilogue (DVE) overlaps the cc1 matmuls
                    nc.vector.tensor_scalar_mul(out_t[:, ts(0, 512)],
                                                pso[0][:], recips[:, i:i + 1])
                    nc.sync.dma_start(out_ext[ts(i, 128), ts(0, 512)],
                                      out_t[:, ts(0, 512)])
                    for j in range(NJT):
                        nc.tensor.matmul(
                            pso[1][:], es[:, j, ts(i, 128)], vs[:, j, ts(1, 512)],
                            start=(j == 0), stop=(j == NJT - 1))
                    if i < NIB - 1:
                        nc.scalar.mul(out_t[:, ts(1, 512)], pso[1][:],
                                      recips[:, i:i + 1])
                    else:
                        # final epilogue: split across DVE+ACT to cut the tail
                        nc.vector.tensor_scalar_mul(
                            out_t[:, ds(512, 256)], pso[1][:, ds(0, 256)],
                            recips[:, i:i + 1])
                        nc.scalar.mul(out_t[:, ds(768, 256)],
                                      pso[1][:, ds(256, 256)],
                                      recips[:, i:i + 1])
                    nc.sync.dma_start(out_ext[ts(i, 128), ts(1, 512)],
                                      out_t[:, ts(1, 512)])

    nc.compile()
    return nc


_NC_CACHE = {}


def _get_nc(qk_bias, v_bias):
    key = (qk_bias, v_bias)
    if key not in _NC_CACHE:
        _NC_CACHE[key] = build_kernel(qk_bias=qk_bias, v_bias=v_bias)
    return _NC_CACHE[key]


def prep_in_maps(x, position, ln_gamma, ln_beta, W_qkv, b_qkv):
    """Host-side sharding / layout prep. Returns in_maps for 8 cores."""
    x = np.asarray(x, dtype=np.float32)
    position = np.asarray(position, dtype=np.float32)
    ln_gamma = np.asarray(ln_gamma, dtype=np.float32)
    ln_beta = np.asarray(ln_beta, dtype=np.float32)
    W_qkv = np.asarray(W_qkv, dtype=np.float32)
    b_qkv = np.asarray(b_qkv, dtype=np.float32)

    # Fold gamma into W columns, beta into bias; fold SCALE into q slice.
    Wp = W_qkv * ln_gamma[None, :]
    bp = b_qkv + W_qkv @ ln_beta
    Wp[:C] *= SCALE
    bp[:C] *= SCALE
    w_t = np.ascontiguousarray(Wp.T).astype(ml_dtypes.bfloat16)  # [C, 3C]
    wsum = np.ascontiguousarray(Wp.astype(ml_dtypes.bfloat16).astype(np.float32).sum(axis=1),
                                dtype=np.float32)

    in_maps = []
    for core in range(8):
        b, s = divmod(core, 2)
        if s == 0:
            x_sh = x[b]
            pos_rot = position
        else:
            x_sh = np.roll(x[b], -MY, axis=1)
            pos_rot = np.roll(position, -MY, axis=1)
        pos_t = np.ascontiguousarray(pos_rot[s * MY:(s + 1) * MY, :].T)  # [N, MY]
        in_maps.append({
            "x_sh": np.ascontiguousarray(x_sh).astype(ml_dtypes.bfloat16),
            "w_t": w_t,
            "bias": bp,
            "wsum": wsum,
            "pos_t": pos_t.astype(ml_dtypes.bfloat16),
        })
    return in_maps


def kernel(x, position, ln_gamma, ln_beta, W_qkv, b_qkv):
    in_maps = prep_in_maps(x, position, ln_gamma, ln_beta, W_qkv, b_qkv)
    bp = in_maps[0]["bias"]
    nc = _get_nc(bool(np.abs(bp[:2 * C]).max() > 0),
                 bool(np.abs(bp[2 * C:]).max() > 0))
    res = run_bass_kernel_spmd(nc, in_maps, core_ids=list(range(8)))
    out = np.empty((B, C, N), dtype=np.float32)
    for core in range(8):
        b, s = divmod(core, 2)
        out[b, :, s * MY:(s + 1) * MY] = res.results[core]["out"].astype(np.float32).T
    return out


# revision 44
# speedup vs baseline: 1.0117x; 1.0117x over previous
"""Trainium2 Bass kernel for nn_Attention_54030688584207.

Single-head attention block:
    h = LN(x^T) ; qkv = h @ W^T + b ; S = q k^T / sqrt(N) + position
    out = softmax(S) @ v, returned as [B, C, N].

Sharding: 8 cores = 4 batches x 2 query-halves, no collectives. Each core
receives its batch's x rotated so its own 1024 query tokens come first and
computes q for its half plus full K/V for the batch (K/V replicated within
the pair), then scores/softmax/PV for its 1024 query rows.

LayerNorm is folded into the QKV epilogues instead of materializing h:
    qkv[d,n] = rstd[n]*( (W'x)[d,n] - mu[n]*wsum[d] ) + b'[d]
so all projection matmuls run on raw (bf16) x with no LN dependency. The
LN statistics are computed entirely off the tensor engine: per-512-token
chunk, the 8 channel-chunks of x (and of x^2 from ScalarE squares) are
pair-summed on the DVE (bf16, multi-dim APs, in place), then reduced
across the 128 partitions with a GpSimd partition_all_reduce, giving
full-width [128,512] sums with no matmul and no broadcast. rstd comes
from DVE reciprocal + ScalarE Sqrt (no Ln -> no activation-table thrash).

Softmax skips max-subtraction (scores are O(5), safe in f32/bf16) so
exp(S^T) feeds PV directly as the stationary operand; row sums accumulate
in a single PSUM bank ([128,8], one column per query block) via 1-wide
matmuls folded into phase C, so phase D starts with all reciprocals ready
and the kernel tail is one epilogue + DMA.

Device layouts (per core):
    x_sh  [C=1024, N=2048] bf16  channels x tokens (token-rotated)
    w_t   [C=1024, 3C=3072] bf16 W'^T (gamma/SCALE folded on host)
    bias  [3072] f32             b' (beta folded, q-part scaled)
    pos_t [N=2048, MY=1024] bf16 position^T (rows in local key order)
    out   [MY=1024, C=1024] f32  out[i, c]  (host transposes back)
"""

import os
import sys

for _p in ("/opt/trn_rl_repo",):
    if _p not in sys.path and os.path.isdir(_p):
        sys.path.insert(0, _p)

import numpy as np
import ml_dtypes

import concourse.bass as bass
import concourse.bass_isa as bass_isa
import concourse.tile as tile
from concourse import bacc, mybir
from concourse.bass import ts, ds
from concourse.bass_utils import run_bass_kernel_spmd

FP = mybir.dt.float32
BF = mybir.dt.bfloat16
AF = mybir.ActivationFunctionType

B = 4
C = 1024
N = 2048
MY = 1024  # query rows per core
D3 = 3 * C
NCH = C // 128   # 8 channel chunks
NJT = N // 128   # 16 key tiles
NIB = MY // 128  # 8 query blocks
NTC = N // 512   # 4 token chunks
LN_EPS = 1e-5
SCALE = 1.0 / np.sqrt(N)


def build_kernel(rep=1, qk_bias=False, v_bias=False):
    nc = bacc.Bacc("TRN2", target_bir_lowering=False, debug=False, num_devices=8)
    x_ext = nc.declare_dram_parameter("x_sh", [C, N], BF, isOutput=False)
    wt_ext = nc.declare_dram_parameter("w_t", [C, D3], BF, isOutput=False)
    b_ext = nc.declare_dram_parameter("bias", [D3], FP, isOutput=False)
    ws_ext = nc.declare_dram_parameter("wsum", [D3], FP, isOutput=False)
    pos_ext = nc.declare_dram_parameter("pos_t", [N, MY], BF, isOutput=False)
    out_ext = nc.declare_dram_parameter("out", [MY, C], BF, isOutput=True)

    x_r = x_ext.ap().rearrange("(a p) n -> p a n", p=128)      # [128, 8, N]
    wt_r = wt_ext.ap().rearrange("(a p) d -> p a d", p=128)    # [128, 8, D3]
    b_r = b_ext.ap().rearrange("(a p) -> p a", p=128)          # [128, 24]
    ws_r = ws_ext.ap().rearrange("(a p) -> p a", p=128)        # [128, 24]

    with tile.TileContext(nc) as tc:
      for _r in range(rep):
        with (
            tc.tile_pool(name=f"res{_r}", bufs=1) as res,
            tc.tile_pool(name=f"statb{_r}", bufs=2) as statb,
            tc.tile_pool(name=f"pospool{_r}", bufs=2) as pospool,
            tc.tile_pool(name=f"xsqp{_r}", bufs=1) as xsqp,
            tc.tile_pool(name=f"treep{_r}", bufs=1) as treep,
            tc.tile_pool(name=f"sump{_r}", bufs=2) as sump,
            tc.tile_pool(name=f"scr{_r}", bufs=3) as scr,
            tc.tile_pool(name=f"rows{_r}", bufs=1) as rows,
            tc.tile_pool(name=f"dramp{_r}", bufs=1, space="DRAM") as dramp,
            tc.tile_pool(name=f"psum{_r}", bufs=1, space="PSUM") as psum,
        ):
            # ---- resident tiles ----
            xh = res.tile([128, NCH, N], BF, tag="big")       # raw x (bf16)
            qs = res.tile([128, NCH, MY], BF, tag="qs")       # q^T  [c, i]
            ks = res.tile([128, NCH, N], BF, tag="ks")        # k^T  [c, j]
            vs = res.tile([128, NJT, C], BF, tag="vs")        # v    [j, c]
            wqk = res.tile([128, NCH, 2 * C], BF, tag="wqk")  # W'^T q,k cols
            wv = res.tile([128, NCH, C], BF, tag="wv")        # W'^T v cols

            ones_b = rows.tile([128, 1], BF, tag="ones_b")
            nc.vector.memset(ones_b[:], 1.0)

            # LN stat tiles (bf16, full width): -mu*rstd and rstd per token
            nmr_b = statb.tile([128, N], BF, tag="statmb", name="nmr_b")
            rstd_b = statb.tile([128, N], BF, tag="statmb", name="rstd_b")
            # per-token-tile columns for the v epilogue (via DRAM bounce):
            # -mu and +rstd (f32)
            nmu_col = rows.tile([128, NJT], FP, tag="nmu_col")
            rstd_col = rows.tile([128, NJT], FP, tag="rstd_col")
            nmu_dram = dramp.tile([1, N], FP, tag="nmu_dram")
            rstd_dram = dramp.tile([1, N], FP, tag="rstd_dram")

            # ---- load x and weights (x t0 first: stats matmuls chase it) ----
            for ch in range(4):
                nc.sync.dma_start(xh[:, ds(ch * 2, 2), ts(0, 512)],
                                  x_r[:, ds(ch * 2, 2), ts(0, 512)])
            nc.sync.dma_start(wqk[:, :, ds(0, 256)], wt_r[:, :, ds(0, 256)])
            nc.sync.dma_start(wqk[:, :, ds(256, 256)], wt_r[:, :, ds(256, 256)])
            bias_sb = rows.tile([128, 24], FP, tag="bias")
            nc.sync.dma_start(bias_sb[:], b_r)
            wsum_sb = rows.tile([128, 24], FP, tag="wsum")
            nc.sync.dma_start(wsum_sb[:], ws_r)
            nc.sync.dma_start(wqk[:, :, ds(512, 512)], wt_r[:, :, ds(512, 512)])
            nc.sync.dma_start(xh[:, :, ts(1, 512)], x_r[:, :, ts(1, 512)])
            nc.sync.dma_start(xh[:, :, ts(2, 512)], x_r[:, :, ts(2, 512)])
            nc.sync.dma_start(xh[:, :, ts(3, 512)], x_r[:, :, ts(3, 512)])
            for piece in range(2):
                nc.sync.dma_start(wqk[:, :, ds(C + piece * 512, 512)],
                                  wt_r[:, :, ds(C + piece * 512, 512)])
            nc.sync.dma_start(wv[:], wt_r[:, :, ds(2 * C, C)])

            # v-weight-colsum (+opt bias) broadcast rows [1, C] -> [128, C]
            wvrow = statb.tile([1, C], BF, tag="statb", name="wvrow")
            nc.gpsimd.dma_start(wvrow[:], ws_ext.ap()[ds(2 * C, C)].rearrange("(o c) -> o c", o=1))
            wvsum_b = rows.tile([128, C], BF, tag="wvsb")
            nc.gpsimd.partition_broadcast(wvsum_b[:], wvrow[:])
            if v_bias:
                bvrow = statb.tile([1, C], BF, tag="statb", name="bvrow")
                nc.gpsimd.dma_start(bvrow[:], b_ext.ap()[ds(2 * C, C)].rearrange("(o c) -> o c", o=1))
                bv_b = rows.tile([128, C], BF, tag="bvb")
                nc.gpsimd.partition_broadcast(bv_b[:], bvrow[:])

            # ---- Phase A: LN stats per 512-token chunk ----
            # t0 on the PE (ones-matmuls chase the x DMA and warm the ramp);
            # t1-3 off the PE: x pair-sum tree on DVE, x^2 tree on GpSimd,
            # then a partition_all_reduce gives full-width sums directly.
            def stats_cols(t, nrow, rrow):
                    nc.sync.dma_start(nmu_dram[0:1, ts(t, 512)], nrow[:])
                    nc.sync.dma_start(rstd_dram[0:1, ts(t, 512)], rrow[:])
                    nc.sync.dma_start(
                        nmu_col[:, ds(t * 4, 4)],
                        nmu_dram[0:1, ts(t, 512)].rearrange("o (f p) -> (o p) f", p=128))
                    nc.sync.dma_start(
                        rstd_col[:, ds(t * 4, 4)],
                        rstd_dram[0:1, ts(t, 512)].rearrange("o (f p) -> (o p) f", p=128))

            def stats_rows_psum(t, sx_row, sq_ap):
                    # [1,512] row chain from PSUM sums, then Pool broadcasts:
                    # nmu = -sx/C ; var = C*var_true = sq - C*nmu^2
                    # rstd = sqrt(C * 1/var)
                    nrow = scr.tile([1, 512], FP, tag="row", bufs=2,
                                    name=f"nrow{t}")
                    nc.scalar.mul(nrow[:], sx_row[:], -1.0 / C)
                    var = scr.tile([1, 512], FP, tag="var", bufs=1,
                                   name=f"varr{t}")
                    nc.vector.tensor_mul(var[:], nrow[:], nrow[:])
                    nc.vector.scalar_tensor_tensor(
                        var[:], var[:], -float(C), sq_ap,
                        op0=mybir.AluOpType.mult, op1=mybir.AluOpType.add)
                    nc.vector.reciprocal(var[:], var[:])
                    rrow = scr.tile([1, 512], FP, tag="row", bufs=2,
                                    name=f"rrow{t}")
                    nc.scalar.activation(rrow[:], var[:], AF.Sqrt,
                                         scale=float(C))
                    rstd_cb = scr.tile([1, 512], BF, tag="cb", bufs=2,
                                       name=f"rstd_cb{t}")
                    nc.vector.tensor_copy(rstd_cb[:], rrow[:])
                    nmr_cb = scr.tile([1, 512], BF, tag="cb", bufs=2,
                                      name=f"nmr_cb{t}")
                    nc.vector.tensor_mul(nmr_cb[:], nrow[:], rrow[:])
                    nc.gpsimd.partition_broadcast(rstd_b[:, ts(t, 512)],
                                                  rstd_cb[:])
                    nc.gpsimd.partition_broadcast(nmr_b[:, ts(t, 512)],
                                                  nmr_cb[:])
                    stats_cols(t, nrow, rrow)

            def stats_rows_sbuf(t, sx_row, sq_row):
                    # full-width chain on [128,512] SBUF sums (no broadcast):
                    # var = reciprocal(sx*sx/C - sq) = -1/(C*var_true)
                    # rstd = sqrt(var * -C) ; nmr = (sx * -1/C) * rstd
                    var = scr.tile([128, 512], FP, tag="var", bufs=1,
                                   name=f"var{t}")
                    nc.vector.tensor_mul(var[:], sx_row[:], sx_row[:])
                    nc.vector.scalar_tensor_tensor(
                        var[:], var[:], 1.0 / C, sq_row[:],
                        op0=mybir.AluOpType.mult, op1=mybir.AluOpType.subtract)
                    nc.vector.reciprocal(var[:], var[:])
                    nc.scalar.activation(rstd_b[:, ts(t, 512)], var[:],
                                         AF.Sqrt, scale=-float(C))
                    nc.vector.scalar_tensor_tensor(
                        nmr_b[:, ts(t, 512)], sx_row[:], -1.0 / C,
                        rstd_b[:, ts(t, 512)],
                        op0=mybir.AluOpType.mult, op1=mybir.AluOpType.mult)
                    nrow = scr.tile([1, 512], FP, tag="row", bufs=2,
                                    name=f"nrow{t}")
                    nc.scalar.mul(nrow[:], sx_row[0:1, :], -1.0 / C)
                    rrow = scr.tile([1, 512], FP, tag="row", bufs=2,
                                    name=f"rrow{t}")
                    nc.scalar.activation(rrow[:], var[0:1, :], AF.Sqrt,
                                         scale=-float(C))
                    stats_cols(t, nrow, rrow)

            def stats_chunk_pe(t):
                    # both column sums via PE ones-matmuls: they chase the x
                    # DMA, anchor the tensor engine's p-state ramp, and keep
                    # the t0 stats entirely off the DVE (whose budget in the
                    # projection window is the binding constraint).
                    qt = treep.tile([128, 4, 512], BF, tag="qt", name=f"qt{t}")
                    xsq = xsqp.tile([128, 4, 512], BF, tag="xsq", name=f"xsq{t}")
                    for c in range(4):
                        nc.scalar.square(qt[:, ds(c, 1), :],
                                         xh[:, ds(c, 1), ts(t, 512)])
                    for c in range(4):
                        nc.scalar.square(xsq[:, ds(c, 1), :],
                                         xh[:, ds(c + 4, 1), ts(t, 512)])
                    ps_s = psum.tile([1, 512], FP, tag="w", bufs=7, name=f"ps_s{t}")
                    ps_q = psum.tile([1, 512], FP, tag="w", bufs=7, name=f"ps_q{t}")
                    for c in range(NCH):
                        nc.tensor.matmul(ps_s[:], ones_b[:], xh[:, c, ts(t, 512)],
                                         start=(c == 0), stop=(c == NCH - 1))
                    for c in range(NCH):
                        src = qt[:, c, :] if c < 4 else xsq[:, c - 4, :]
                        nc.tensor.matmul(ps_q[:], ones_b[:], src,
                                         start=(c == 0), stop=(c == NCH - 1))
                    stats_rows_psum(t, ps_s, ps_q[:])

            def stats_sums_x(t):
                    xt = treep.tile([128, 4, 512], BF, tag="xt", name=f"xt{t}")
                    nc.vector.tensor_add(xt[:], xh[:, ds(0, 4), ts(t, 512)],
                                         xh[:, ds(4, 4), ts(t, 512)])
                    nc.vector.tensor_add(xt[:, ds(0, 2), :], xt[:, ds(0, 2), :],
                                         xt[:, ds(2, 2), :])
                    sum_x = sump.tile([128, 512], FP, tag="sum_x", bufs=1,
                                      name=f"sum_x{t}")
                    nc.vector.tensor_add(sum_x[:], xt[:, 0, :], xt[:, 1, :])
                    nc.gpsimd.partition_all_reduce(sum_x[:], sum_x[:], 128,
                                                   bass_isa.ReduceOp.add)
                    return sum_x

            def stats_sums_q(t):
                    qt = treep.tile([128, 4, 512], BF, tag="qt", name=f"qt{t}")
                    xsq = xsqp.tile([128, 4, 512], BF, tag="xsq", name=f"xsq{t}")
                    for c in range(4):
                        nc.scalar.square(qt[:, ds(c, 1), :],
                                         xh[:, ds(c, 1), ts(t, 512)])
                    for c in range(4):
                        nc.scalar.square(xsq[:, ds(c, 1), :],
                                         xh[:, ds(c + 4, 1), ts(t, 512)])
                    nc.vector.tensor_add(qt[:], qt[:], xsq[:])
                    nc.vector.tensor_add(qt[:, ds(0, 2), :], qt[:, ds(0, 2), :],
                                         qt[:, ds(2, 2), :])
                    sum_q = sump.tile([128, 512], FP, tag="sum_q", bufs=1,
                                      name=f"sum_q{t}")
                    nc.vector.tensor_add(sum_q[:], qt[:, 0, :], qt[:, 1, :])
                    nc.gpsimd.partition_all_reduce(sum_q[:], sum_q[:], 128,
                                                   bass_isa.ReduceOp.add)
                    return sum_q

            def stats_chunk(t):
                    sum_x = stats_sums_x(t)
                    sum_q = stats_sums_q(t)
                    stats_rows_sbuf(t, sum_x, sum_q)

            # ---- Phase B1: q^T and k^T (weights stationary, c-outer groups) ----
            def qk_group(dts, tlist):
                    pss = {}
                    for dt in dts:
                        for t in tlist:
                            pss[(dt, t)] = psum.tile([128, 512], FP, tag="w",
                                                     bufs=7, name=f"qkv_{dt}_{t}")
                    for c in range(NCH):
                        for dt in dts:
                            for t in tlist:
                                nc.tensor.matmul(
                                    pss[(dt, t)][:], wqk[:, c, ts(dt, 128)],
                                    xh[:, c, ts(t, 512)],
                                    start=(c == 0), stop=(c == NCH - 1))
                    for dt in dts:
                        for t in tlist:
                            # t1 = G + (-mu*rstd)*wsum[d] ; qk = t1 * rstd
                            t1 = scr.tile([128, 512], BF, tag="t1", bufs=2,
                                          name=f"t1_{dt}_{t}")
                            nc.vector.scalar_tensor_tensor(
                                t1[:], nmr_b[:, ts(t, 512)], wsum_sb[:, dt:dt + 1],
                                pss[(dt, t)][:],
                                op0=mybir.AluOpType.mult, op1=mybir.AluOpType.add)
                            dst = (qs[:, dt, ts(t, 512)] if dt < 8
                                   else ks[:, dt - 8, ts(t, 512)])
                            nc.vector.tensor_mul(dst, t1[:], rstd_b[:, ts(t, 512)])
                            if qk_bias:
                                nc.vector.tensor_scalar_add(
                                    dst, dst, bias_sb[:, dt:dt + 1])

            stats_chunk_pe(0)
            qk_group(range(0, 2), [0])
            qk_group(range(2, 4), [0])
            stats_parts = {}
            stats_parts["x1"] = stats_sums_x(1)
            qk_group(range(4, 6), [0])
            stats_parts["q1"] = stats_sums_q(1)
            qk_group(range(6, 8), [0])
            stats_rows_sbuf(1, stats_parts["x1"], stats_parts["q1"])
            for g in range(0, 8, 2):
                qk_group(range(g, g + 2), [1])

            # ---- Phase B2: v (activations stationary) ----
            def v_group(jts):
                for jt in jts:
                    for cc in range(C // 512):
                        psv = psum.tile([128, 512], FP, tag="w",
                                        bufs=7, name=f"psv_{jt}_{cc}")
                        for c in range(NCH):
                            nc.tensor.matmul(
                                psv[:], xh[:, c, ts(jt, 128)],
                                wv[:, c, ts(cc, 512)],
                                start=(c == 0), stop=(c == NCH - 1))
                        # t1 = Gv + wvsum*(-mu[n]) ; v = t1*rstd[n] (+ bv)
                        t1v = scr.tile([128, 512], BF, tag="t1", bufs=2,
                                       name=f"t1v_{jt}_{cc}")
                        nc.vector.scalar_tensor_tensor(
                            t1v[:], wvsum_b[:, ts(cc, 512)], nmu_col[:, jt:jt + 1],
                            psv[:],
                            op0=mybir.AluOpType.mult, op1=mybir.AluOpType.add)
                        if v_bias:
                            nc.vector.scalar_tensor_tensor(
                                vs[:, jt, ts(cc, 512)], t1v[:],
                                rstd_col[:, jt:jt + 1], bv_b[:, ts(cc, 512)],
                                op0=mybir.AluOpType.mult, op1=mybir.AluOpType.add)
                        else:
                            nc.scalar.mul(vs[:, jt, ts(cc, 512)], t1v[:],
                                          rstd_col[:, jt:jt + 1])

            # k for tokens 0:1024 first (epilogues need only stats 0/1);
            # the stats 2/3 DVE bursts spread across this PE work, then
            # k for tokens 1024:2048 and v interleave.
            for g in range(8, 16):
                qk_group(range(g, g + 1), [0, 1])
                if g == 8:
                    stats_parts["x2"] = stats_sums_x(2)
                elif g == 9:
                    stats_parts["q2"] = stats_sums_q(2)
                elif g == 10:
                    stats_rows_sbuf(2, stats_parts["x2"], stats_parts["q2"])
                elif g == 11:
                    stats_parts["x3"] = stats_sums_x(3)
                elif g == 12:
                    stats_parts["q3"] = stats_sums_q(3)
                elif g == 13:
                    stats_rows_sbuf(3, stats_parts["x3"], stats_parts["q3"])
            for gi, g in enumerate(range(8, 16)):
                qk_group(range(g, g + 1), [2, 3])
                if g % 2 == 1:
                    v_group(range((g - 9) // 2 * 4, (g - 9) // 2 * 4 + 4))

            # ---- Phase C: S^T = k^T.T q^T + pos ; exp -> es (bf16) ----
            # row sums fold in as 1-wide matmuls into one PSUM bank [128, 8]
            es = res.tile([128, NJT, MY], BF, tag="big")  # reuses xh slot
            ps_sums = psum.tile([128, NIB], FP, tag="sums", bufs=1,
                                name="ps_sums")
            for j in range(NJT):
                    pos_tile = pospool.tile([128, MY], BF, tag="pos")
                    nc.sync.dma_start(pos_tile[:], pos_ext[ts(j, 128), :])
                    psS = [psum.tile([128, 512], FP, tag="w", bufs=7,
                                     name=f"psS{j}_{ih}") for ih in range(2)]
                    for c in range(NCH):
                        for ih in range(MY // 512):
                            nc.tensor.matmul(
                                psS[ih][:], ks[:, c, ts(j, 128)],
                                qs[:, c, ts(ih, 512)],
                                start=(c == 0), stop=(c == NCH - 1))
                    if j > 0:
                        # single start=True: PSUM "start" begins the whole
                        # 2KB zero-region; later writes to untouched elements
                        # overwrite via per-element has_written
                        for i in range(NIB):
                            nc.tensor.matmul(
                                ps_sums[:, i:i + 1], es[:, j - 1, ts(i, 128)],
                                ones_b[:], start=(j == 1 and i == 0),
                                stop=False)
                    for ih in range(2):
                        nc.vector.tensor_add(psS[ih][:], psS[ih][:],
                                             pos_tile[:, ts(ih, 512)])
                        nc.scalar.activation(es[:, j, ts(ih, 512)], psS[ih][:],
                                             AF.Exp)

            # ---- Phase D: out[i, c] = (P^T)^T v / rowsum ----
            recips = rows.tile([128, NIB], FP, tag="recips")
            for i in range(NIB):
                    pso = [psum.tile([128, 512], FP, tag="w", bufs=7,
                                     name=f"pso{i}_{cc}") for cc in range(2)]
                    for j in range(NJT):
                        nc.tensor.matmul(
                            pso[0][:], es[:, j, ts(i, 128)], vs[:, j, ts(0, 512)],
                            start=(j == 0), stop=(j == NJT - 1))
                    if i == 0:
                        # last rowsum tile (es j=15), then all reciprocals
                        for ii in range(NIB):
                            nc.tensor.matmul(
                                ps_sums[:, ii:ii + 1],
                                es[:, NJT - 1, ts(ii, 128)],
                                ones_b[:], start=False, stop=(ii == NIB - 1))
                        nc.vector.reciprocal(recips[:], ps_sums[:])
                    out_t = statb.tile([128, C], BF, tag="statb", name=f"out_t{i}")
                    # cc0 epilogue (DVE) overlaps the cc1 matmuls
                    nc.vector.tensor_scalar_mul(out_t[:, ts(0, 512)],
                                                pso[0][:], recips[:, i:i + 1])
                    nc.sync.dma_start(out_ext[ts(i, 128), ts(0, 512)],
                                      out_t[:, ts(0, 512)])
                    for j in range(NJT):
                        nc.tensor.matmul(
                            pso[1][:], es[:, j, ts(i, 128)], vs[:, j, ts(1, 512)],
                            start=(j == 0), stop=(j == NJT - 1))
                    nc.scalar.mul(out_t[:, ts(1, 512)], pso[1][:],
                                  recips[:, i:i + 1])
                    nc.sync.dma_start(out_ext[ts(i, 128), ts(1, 512)],
                                      out_t[:, ts(1, 512)])

    nc.compile()
    return nc


_NC_CACHE = {}


def _get_nc(qk_bias, v_bias):
    key = (qk_bias, v_bias)
    if key not in _NC_CACHE:
        _NC_CACHE[key] = build_kernel(qk_bias=qk_bias, v_bias=v_bias)
    return _NC_CACHE[key]


def prep_in_maps(x, position, ln_gamma, ln_beta, W_qkv, b_qkv):
    """Host-side sharding / layout prep. Returns in_maps for 8 cores."""
    x = np.asarray(x, dtype=np.float32)
    position = np.asarray(position, dtype=np.float32)
    ln_gamma = np.asarray(ln_gamma, dtype=np.float32)
    ln_beta = np.asarray(ln_beta, dtype=np.float32)
    W_qkv = np.asarray(W_qkv, dtype=np.float32)
    b_qkv = np.asarray(b_qkv, dtype=np.float32)

    # Fold gamma into W columns, beta into bias; fold SCALE into q slice.
    Wp = W_qkv * ln_gamma[None, :]
    bp = b_qkv + W_qkv @ ln_beta
    Wp[:C] *= SCALE
    bp[:C] *= SCALE
    w_t = np.ascontiguousarray(Wp.T).astype(ml_dtypes.bfloat16)  # [C, 3C]
    wsum = np.ascontiguousarray(Wp.astype(ml_dtypes.bfloat16).astype(np.float32).sum(axis=1),
                                dtype=np.float32)

    in_maps = []
    for core in range(8):
        b, s = divmod(core, 2)
        if s == 0:
            x_sh = x[b]
            pos_rot = position
        else:
            x_sh = np.roll(x[b], -MY, axis=1)
            pos_rot = np.roll(position, -MY, axis=1)
        pos_t = np.ascontiguousarray(pos_rot[s * MY:(s + 1) * MY, :].T)  # [N, MY]
        in_maps.append({
            "x_sh": np.ascontiguousarray(x_sh).astype(ml_dtypes.bfloat16),
            "w_t": w_t,
            "bias": bp,
            "wsum": wsum,
            "pos_t": pos_t.astype(ml_dtypes.bfloat16),
        })
    return in_maps


def kernel(x, position, ln_gamma, ln_beta, W_qkv, b_qkv):
    in_maps = prep_in_maps(x, position, ln_gamma, ln_beta, W_qkv, b_qkv)
    bp = in_maps[0]["bias"]
    nc = _get_nc(bool(np.abs(bp[:2 * C]).max() > 0),
                 bool(np.abs(bp[2 * C:]).max() > 0))
    res = run_bass_kernel_spmd(nc, in_maps, core_ids=list(range(8)))
    out = np.empty((B, C, N), dtype=np.float32)
    for core in range(8):
        b, s = divmod(core, 2)
        out[b, :, s * MY:(s + 1) * MY] = res.results[core]["out"].astype(np.float32).T
    return out
